# revision 7
# baseline (speedup 1.0000x reference)
"""Trainium2 Bass kernel for nn_KnowledgeGraphGNN (8-node complete-graph GCN over a batch).

Math (exact algebra, valid for any inputs):
  w[b,:]  = softmax(latent[b,:8]);  X[b,n,f] = NF[n,f] + 0.1*w[b,n]
  X@W1    = C1[n,h] + 0.1*w[b,n]*s1[h]          (C1 = NF@W1, s1 = colsum W1)
  z[b,i,h]= D1[i,h] + b1[h] + 0.1*U[b,i]*s1[h]  (D1 = A_hat@C1, U = A_hat@w[b])
  out[b,:]= sum_{i,h} relu(z)[b,i,h] * c[i]*W2[h,:] + b2   (c = colmean of A_hat)

Softmax normalization is folded through the ReLU (positive homogeneity):
with E = exp(latent[:, :8]) and S = sum_j E,
  S*z[b,i,h] = sum_j (d1[i,h] + 0.1*s1[h]*A_hat[i,j]) * E[b,j]
so each hidden tile is a K=8 matmul of E^T against a host-folded constant,
then ReLU, then a K=2048 matmul against c[i]*W2, then a 1/S per-row scale
(b2 rides along as an extra K=8 matmul contributing S[b]*b2[o]).

Sharding: pure data-parallel, batch 8192 -> 8 cores x 1024.
"""

import os
import numpy as np

B, NNODE, FDIM, HDIM, ODIM = 8192, 8, 512, 256, 128
NCORES = 8
BC = B // NCORES          # 1024 batch rows per core
NCHUNK = 16               # h-chunks: partitions hold p = i*16 + (h % 16), h = cc*16 + p%16
HALF = 512                # fp32 matmul max moving free dim
KPAD = 32                 # contraction dim for the K=8 matmuls, zero-padded to 32

_CACHE = {}
LAST_RESULTS = None       # BassKernelResults of the most recent run (for profiling)


def _build_nc():
    import concourse.bacc as bacc
    import concourse.mybir as mybir
    from concourse.tile import TileContext
    from concourse._compat import get_trn_type

    fp32 = mybir.dt.float32
    AF = mybir.ActivationFunctionType

    nc = bacc.Bacc(get_trn_type() or "TRN2", target_bir_lowering=False, debug=True)

    d_latT = nc.dram_tensor("latT", [NNODE, BC], fp32, kind="ExternalInput")
    d_lat8 = nc.dram_tensor("lat8", [BC, NNODE], fp32, kind="ExternalInput")
    d_zlhs = nc.dram_tensor("zlhs", [KPAD, NCHUNK, 128], fp32, kind="ExternalInput")
    d_w2pk = nc.dram_tensor("w2pk", [128, NCHUNK, ODIM], fp32, kind="ExternalInput")
    d_b2r = nc.dram_tensor("b2r", [KPAD, ODIM], fp32, kind="ExternalInput")
    d_out = nc.dram_tensor("out", [BC, ODIM], fp32, kind="ExternalOutput")

    with TileContext(nc) as tc:
        with (
            tc.tile_pool(name="consts", bufs=1) as consts,
            tc.tile_pool(name="work", bufs=2) as work,
            tc.tile_pool(name="hbuf", bufs=1) as hbuf,
            tc.tile_pool(name="outsb", bufs=3) as outsb,
            tc.tile_pool(name="zpsum", bufs=4, space="PSUM") as zpsum,
            tc.tile_pool(name="opsum", bufs=3, space="PSUM") as opsum,
        ):
            # ---- constants / inputs ----
            sb_zlhs = consts.tile([KPAD, NCHUNK, 128], fp32)
            nc.sync.dma_start(out=sb_zlhs[:], in_=d_zlhs[:])
            sb_w2 = consts.tile([128, NCHUNK, ODIM], fp32)
            nc.sync.dma_start(out=sb_w2[:], in_=d_w2pk[:])
            sb_b2r = consts.tile([KPAD, ODIM], fp32)
            nc.sync.dma_start(out=sb_b2r[:], in_=d_b2r[:])

            sb_latT = work.tile([NNODE, BC], fp32)
            nc.sync.dma_start(out=sb_latT[:], in_=d_latT[:])
            # batch-on-partition copy: [128, t, j] with b = t*128 + p
            sb_lat8 = work.tile([128, BC // 128, NNODE], fp32)
            nc.sync.dma_start(
                out=sb_lat8[:], in_=d_lat8.rearrange("(t p) j -> p t j", p=128)
            )

            # ---- softmax pieces ----
            # E^T = exp(latT)   [8, BC] on partitions 0:8; rows 8:KPAD zero
            # (matmuls with <32 contraction partitions fault on HW, so pad K)
            sb_ET = work.tile([KPAD, BC], fp32)
            nc.vector.memset(sb_ET[:], 0.0)
            nc.scalar.activation(out=sb_ET[:NNODE, :], in_=sb_latT[:], func=AF.Exp)
            # E2 = exp(lat8)    [128, t, j] ; S2 = sum_j ; R2 = 1/S2   [128, t]
            sb_E2 = work.tile([128, BC // 128, NNODE], fp32)
            nc.scalar.activation(out=sb_E2[:], in_=sb_lat8[:], func=AF.Exp)
            sb_S2 = work.tile([128, BC // 128], fp32)
            nc.vector.reduce_sum(out=sb_S2[:], in_=sb_E2[:], axis=mybir.AxisListType.X)
            sb_R2 = work.tile([128, BC // 128], fp32)
            nc.vector.reciprocal(out=sb_R2[:], in_=sb_S2[:])

            # ---- hidden: Z_cc = zlhs_cc^T @ E^T  -> relu -> H_cc  [128, BC] ----
            h_tiles = []
            for cc in range(NCHUNK):
                h_sb = hbuf.tile([128, BC], fp32, tag=f"h{cc}")
                for half in range(BC // HALF):
                    z_ps = zpsum.tile([128, HALF], fp32)
                    nc.tensor.matmul(
                        z_ps[:],
                        sb_zlhs[:, cc, :],
                        sb_ET[:, half * HALF : (half + 1) * HALF],
                        start=True,
                        stop=True,
                    )
                    dst = h_sb[:, half * HALF : (half + 1) * HALF]
                    if (cc + half) % 2 == 0:
                        nc.scalar.activation(out=dst, in_=z_ps[:], func=AF.Relu)
                    else:
                        nc.vector.tensor_scalar_max(dst, z_ps[:], 0.0)
                h_tiles.append(h_sb)

            # ---- output: out[bq] = (sum_cc H_cc^T[:, bq128].T @ W2_cc + S*b2) * r ----
            for bq in range(BC // 128):
                o_ps = opsum.tile([128, ODIM], fp32)
                for cc in range(NCHUNK):
                    nc.tensor.matmul(
                        o_ps[:],
                        h_tiles[cc][:, bq * 128 : (bq + 1) * 128],
                        sb_w2[:, cc, :],
                        start=(cc == 0),
                        stop=False,
                    )
                # + S[b] * b2[o]
                nc.tensor.matmul(
                    o_ps[:],
                    sb_ET[:, bq * 128 : (bq + 1) * 128],
                    sb_b2r[:],
                    start=False,
                    stop=True,
                )
                o_sb = outsb.tile([128, ODIM], fp32)
                nc.scalar.activation(
                    out=o_sb[:], in_=o_ps[:], func=AF.Copy,
                    scale=sb_R2[:, bq : bq + 1],
                )
                nc.sync.dma_start(
                    out=d_out[bq * 128 : (bq + 1) * 128, :], in_=o_sb[:]
                )

    nc.finalize()
    return nc


def _host_constants(node_features, edge_attr, W1, b1, W2, b2):
    nf = np.asarray(node_features, np.float32)
    ew = np.asarray(edge_attr, np.float32)[:, 0]
    W1 = np.asarray(W1, np.float32)
    b1 = np.asarray(b1, np.float32)
    W2 = np.asarray(W2, np.float32)
    b2 = np.asarray(b2, np.float32)

    # A_hat = D^-1/2 (A + I) D^-1/2, edges (i, j) for all i != j row-major
    src = np.array([i for i in range(NNODE) for j in range(NNODE) if i != j], np.int64)
    dst = np.array([j for i in range(NNODE) for j in range(NNODE) if i != j], np.int64)
    A = np.zeros((NNODE, NNODE), np.float32)
    A[dst, src] = ew
    A = A + np.eye(NNODE, dtype=np.float32)
    deg = A.sum(axis=1)
    dinv = np.where(deg > 0, deg.astype(np.float32) ** -0.5, 0.0).astype(np.float32)
    A_hat = dinv[:, None] * A * dinv[None, :]

    C1 = nf @ W1                      # [8, 256]
    D1 = A_hat @ C1                   # [8, 256]
    d1 = D1 + b1[None, :]             # [8, 256]
    s1 = W1.sum(axis=0)               # [256]
    cvec = A_hat.mean(axis=0)         # [8]

    p = np.arange(128)
    ip = p // 16                      # node index per partition
    qp = p % 16                       # h sub-index per partition

    # zlhs[j, cc, p] = d1[i(p), h(cc,p)] + 0.1*s1[h(cc,p)]*A_hat[i(p), j]
    zlhs = np.zeros((KPAD, NCHUNK, 128), np.float32)
    for cc in range(NCHUNK):
        h = cc * 16 + qp              # [128]
        zlhs[:NNODE, cc, :] = d1[ip, h][None, :] + 0.1 * s1[h][None, :] * A_hat[ip, :].T

    # w2pk[p, cc, o] = c[i(p)] * W2[h(cc,p), o]
    w2pk = np.empty((128, NCHUNK, ODIM), np.float32)
    for cc in range(NCHUNK):
        h = cc * 16 + qp
        w2pk[:, cc, :] = cvec[ip][:, None] * W2[h, :]

    b2r = np.zeros((KPAD, ODIM), np.float32)
    b2r[:NNODE, :] = b2[None, :]
    return zlhs, w2pk, b2r


def kernel(latent_vec, node_features, edge_attr, W1, b1, W2, b2):
    global LAST_RESULTS
    from concourse.bass_utils import run_bass_kernel_spmd

    if "nc" not in _CACHE:
        _CACHE["nc"] = _build_nc()
    nc = _CACHE["nc"]

    zlhs, w2pk, b2r = _host_constants(node_features, edge_attr, W1, b1, W2, b2)

    lat8 = np.ascontiguousarray(np.asarray(latent_vec, np.float32)[:, :NNODE])
    in_maps = []
    for c in range(NCORES):
        sl = lat8[c * BC : (c + 1) * BC]
        in_maps.append({
            "latT": np.ascontiguousarray(sl.T),
            "lat8": np.ascontiguousarray(sl),
            "zlhs": zlhs,
            "w2pk": w2pk,
            "b2r": b2r,
        })

    trace = bool(int(os.environ.get("GNN_TRACE", "0")))
    kwargs = {}
    if trace:
        kwargs["trace"] = True
        kwargs["trace_cores"] = [int(x) for x in os.environ.get("GNN_TRACE_CORES", "0").split(",")]
    res = run_bass_kernel_spmd(nc, in_maps, core_ids=list(range(NCORES)), **kwargs)
    LAST_RESULTS = res

    out = np.concatenate([res.results[c]["out"] for c in range(NCORES)], axis=0)
    return out


# revision 13
# speedup vs baseline: 2.2178x; 2.2178x over previous
"""Trainium2 Bass kernel for nn_KnowledgeGraphGNN (8-node complete-graph GCN over a batch).

Math (exact algebra, valid for any inputs):
  w[b,:]  = softmax(latent[b,:8]);  X[b,n,f] = NF[n,f] + 0.1*w[b,n]
  X@W1    = C1[n,h] + 0.1*w[b,n]*s1[h]          (C1 = NF@W1, s1 = colsum W1)
  z[b,i,h]= D1[i,h] + b1[h] + 0.1*U[b,i]*s1[h]  (D1 = A_hat@C1, U = A_hat@w[b])
  out[b,:]= sum_{i,h} relu(z)[b,i,h] * c[i]*W2[h,:] + b2   (c = colmean of A_hat)

Softmax normalization is folded through the ReLU (positive homogeneity):
with E = exp(latent[:, :8]) and S = sum_j E,
  S*z[b,i,h] = sum_j (d1[i,h] + 0.1*s1[h]*A_hat[i,j]) * E[b,j]
so each hidden tile is a K=8 matmul of E^T against a host-folded constant,
then ReLU, then a K=2048 matmul against c[i]*W2, then a 1/S per-row scale
(b2 rides along as an extra K=8 matmul contributing S[b]*b2[o]).

Sharding: pure data-parallel, batch 8192 -> 8 cores x 1024.
"""

import os
import numpy as np

B, NNODE, FDIM, HDIM, ODIM = 8192, 8, 512, 256, 128
NCORES = 8
BC = B // NCORES          # 1024 batch rows per core
NCHUNK = 16               # h-chunks: partitions hold p = i*16 + (h % 16), h = cc*16 + p%16
HALF = 512                # fp32 matmul max moving free dim
KPAD = 32                 # contraction dim for the K=8 matmuls, zero-padded to 32

_CACHE = {}
LAST_RESULTS = None       # BassKernelResults of the most recent run (for profiling)


def _build_nc():
    import concourse.bacc as bacc
    import concourse.mybir as mybir
    from concourse.tile import TileContext
    from concourse._compat import get_trn_type

    fp32 = mybir.dt.float32
    bf16 = mybir.dt.bfloat16
    AF = mybir.ActivationFunctionType

    nc = bacc.Bacc(get_trn_type() or "TRN2", target_bir_lowering=False, debug=True)

    d_latT = nc.dram_tensor("latT", [NNODE, BC], fp32, kind="ExternalInput")
    d_lat8 = nc.dram_tensor("lat8", [BC, NNODE], fp32, kind="ExternalInput")
    d_zlhs = nc.dram_tensor("zlhs", [KPAD, NCHUNK, 128], bf16, kind="ExternalInput")
    d_w2pk = nc.dram_tensor("w2pk", [128, NCHUNK, ODIM], bf16, kind="ExternalInput")
    d_b2r = nc.dram_tensor("b2r", [KPAD, ODIM], bf16, kind="ExternalInput")
    d_out = nc.dram_tensor("out", [BC, ODIM], fp32, kind="ExternalOutput")

    with TileContext(nc) as tc:
        with (
            tc.tile_pool(name="consts", bufs=1) as consts,
            tc.tile_pool(name="work", bufs=2) as work,
            tc.tile_pool(name="hbuf", bufs=1) as hbuf,
            tc.tile_pool(name="outsb", bufs=3) as outsb,
            tc.tile_pool(name="zpsum", bufs=4, space="PSUM") as zpsum,
            tc.tile_pool(name="opsum", bufs=3, space="PSUM") as opsum,
        ):
            # ---- constants / inputs ----
            sb_zlhs = consts.tile([KPAD, NCHUNK, 128], bf16)
            nc.sync.dma_start(out=sb_zlhs[:], in_=d_zlhs[:])
            sb_w2 = consts.tile([128, NCHUNK, ODIM], bf16)
            nc.sync.dma_start(out=sb_w2[:], in_=d_w2pk[:])
            sb_b2r = consts.tile([KPAD, ODIM], bf16)
            nc.sync.dma_start(out=sb_b2r[:], in_=d_b2r[:])

            sb_latT = work.tile([NNODE, BC], fp32)
            nc.sync.dma_start(out=sb_latT[:], in_=d_latT[:])
            # batch-on-partition copy: [128, t, j] with b = t*128 + p
            sb_lat8 = work.tile([128, BC // 128, NNODE], fp32)
            nc.sync.dma_start(
                out=sb_lat8[:], in_=d_lat8.rearrange("(t p) j -> p t j", p=128)
            )

            # ---- softmax pieces ----
            # E^T = exp(latT)   [8, BC] on partitions 0:8; rows 8:KPAD zero
            # (matmuls with <32 contraction partitions fault on HW, so pad K)
            sb_ET = work.tile([KPAD, BC], bf16)
            nc.vector.memset(sb_ET[:], 0.0)
            nc.scalar.activation(out=sb_ET[:NNODE, :], in_=sb_latT[:], func=AF.Exp)
            # E2 = exp(lat8)    [128, t, j] ; S2 = sum_j ; R2 = 1/S2   [128, t]
            sb_E2 = work.tile([128, BC // 128, NNODE], fp32)
            nc.scalar.activation(out=sb_E2[:], in_=sb_lat8[:], func=AF.Exp)
            sb_S2 = work.tile([128, BC // 128], fp32)
            nc.vector.reduce_sum(out=sb_S2[:], in_=sb_E2[:], axis=mybir.AxisListType.X)
            sb_R2 = work.tile([128, BC // 128], fp32)
            nc.vector.reciprocal(out=sb_R2[:], in_=sb_S2[:])

            # ---- hidden: Z_cc = zlhs_cc^T @ E^T  -> relu -> H_cc  [128, BC] ----
            h_tiles = []
            for cc in range(NCHUNK):
                h_sb = hbuf.tile([128, BC], bf16, tag=f"h{cc}")
                for half in range(BC // HALF):
                    z_ps = zpsum.tile([128, HALF], fp32)
                    nc.tensor.matmul(
                        z_ps[:],
                        sb_zlhs[:, cc, :],
                        sb_ET[:, half * HALF : (half + 1) * HALF],
                        start=True,
                        stop=True,
                    )
                    dst = h_sb[:, half * HALF : (half + 1) * HALF]
                    if (cc + half) % 2 == 0:
                        nc.scalar.activation(out=dst, in_=z_ps[:], func=AF.Relu)
                    else:
                        nc.vector.tensor_scalar_max(dst, z_ps[:], 0.0)
                h_tiles.append(h_sb)

            # ---- output: out[bq] = (sum_cc H_cc^T[:, bq128].T @ W2_cc + S*b2) * r ----
            for bq in range(BC // 128):
                o_ps = opsum.tile([128, ODIM], fp32)
                for cc in range(NCHUNK):
                    nc.tensor.matmul(
                        o_ps[:],
                        h_tiles[cc][:, bq * 128 : (bq + 1) * 128],
                        sb_w2[:, cc, :],
                        start=(cc == 0),
                        stop=False,
                    )
                # + S[b] * b2[o]
                nc.tensor.matmul(
                    o_ps[:],
                    sb_ET[:, bq * 128 : (bq + 1) * 128],
                    sb_b2r[:],
                    start=False,
                    stop=True,
                )
                o_sb = outsb.tile([128, ODIM], fp32)
                nc.scalar.activation(
                    out=o_sb[:], in_=o_ps[:], func=AF.Copy,
                    scale=sb_R2[:, bq : bq + 1],
                )
                nc.sync.dma_start(
                    out=d_out[bq * 128 : (bq + 1) * 128, :], in_=o_sb[:]
                )

    nc.finalize()
    return nc


def _host_constants(node_features, edge_attr, W1, b1, W2, b2):
    nf = np.asarray(node_features, np.float32)
    ew = np.asarray(edge_attr, np.float32)[:, 0]
    W1 = np.asarray(W1, np.float32)
    b1 = np.asarray(b1, np.float32)
    W2 = np.asarray(W2, np.float32)
    b2 = np.asarray(b2, np.float32)

    # A_hat = D^-1/2 (A + I) D^-1/2, edges (i, j) for all i != j row-major
    src = np.array([i for i in range(NNODE) for j in range(NNODE) if i != j], np.int64)
    dst = np.array([j for i in range(NNODE) for j in range(NNODE) if i != j], np.int64)
    A = np.zeros((NNODE, NNODE), np.float32)
    A[dst, src] = ew
    A = A + np.eye(NNODE, dtype=np.float32)
    deg = A.sum(axis=1)
    dinv = np.where(deg > 0, deg.astype(np.float32) ** -0.5, 0.0).astype(np.float32)
    A_hat = dinv[:, None] * A * dinv[None, :]

    C1 = nf @ W1                      # [8, 256]
    D1 = A_hat @ C1                   # [8, 256]
    d1 = D1 + b1[None, :]             # [8, 256]
    s1 = W1.sum(axis=0)               # [256]
    cvec = A_hat.mean(axis=0)         # [8]

    p = np.arange(128)
    ip = p // 16                      # node index per partition
    qp = p % 16                       # h sub-index per partition

    import ml_dtypes
    bf16 = ml_dtypes.bfloat16

    # zlhs[j, cc, p] = d1[i(p), h(cc,p)] + 0.1*s1[h(cc,p)]*A_hat[i(p), j]
    zlhs = np.zeros((KPAD, NCHUNK, 128), np.float32)
    for cc in range(NCHUNK):
        h = cc * 16 + qp              # [128]
        zlhs[:NNODE, cc, :] = d1[ip, h][None, :] + 0.1 * s1[h][None, :] * A_hat[ip, :].T

    # w2pk[p, cc, o] = c[i(p)] * W2[h(cc,p), o]
    w2pk = np.empty((128, NCHUNK, ODIM), np.float32)
    for cc in range(NCHUNK):
        h = cc * 16 + qp
        w2pk[:, cc, :] = cvec[ip][:, None] * W2[h, :]

    b2r = np.zeros((KPAD, ODIM), np.float32)
    b2r[:NNODE, :] = b2[None, :]
    return zlhs.astype(bf16), w2pk.astype(bf16), b2r.astype(bf16)


def kernel(latent_vec, node_features, edge_attr, W1, b1, W2, b2):
    global LAST_RESULTS
    from concourse.bass_utils import run_bass_kernel_spmd

    if "nc" not in _CACHE:
        _CACHE["nc"] = _build_nc()
    nc = _CACHE["nc"]

    zlhs, w2pk, b2r = _host_constants(node_features, edge_attr, W1, b1, W2, b2)

    lat8 = np.ascontiguousarray(np.asarray(latent_vec, np.float32)[:, :NNODE])
    in_maps = []
    for c in range(NCORES):
        sl = lat8[c * BC : (c + 1) * BC]
        in_maps.append({
            "latT": np.ascontiguousarray(sl.T),
            "lat8": np.ascontiguousarray(sl),
            "zlhs": zlhs,
            "w2pk": w2pk,
            "b2r": b2r,
        })

    trace = bool(int(os.environ.get("GNN_TRACE", "0")))
    kwargs = {}
    if trace:
        kwargs["trace"] = True
        kwargs["trace_cores"] = [int(x) for x in os.environ.get("GNN_TRACE_CORES", "0").split(",")]
    res = run_bass_kernel_spmd(nc, in_maps, core_ids=list(range(NCORES)), **kwargs)
    LAST_RESULTS = res

    out = np.concatenate([res.results[c]["out"] for c in range(NCORES)], axis=0)
    return out


# revision 17
# speedup vs baseline: 2.6101x; 1.1769x over previous
"""Trainium2 Bass kernel for nn_KnowledgeGraphGNN (8-node complete-graph GCN over a batch).

Math (exact algebra, valid for any inputs):
  w[b,:]  = softmax(latent[b,:8]);  X[b,n,f] = NF[n,f] + 0.1*w[b,n]
  X@W1    = C1[n,h] + 0.1*w[b,n]*s1[h]          (C1 = NF@W1, s1 = colsum W1)
  z[b,i,h]= D1[i,h] + b1[h] + 0.1*U[b,i]*s1[h]  (D1 = A_hat@C1, U = A_hat@w[b])
  out[b,:]= sum_{i,h} relu(z)[b,i,h] * c[i]*W2[h,:] + b2   (c = colmean of A_hat)

Softmax normalization is folded through the ReLU (positive homogeneity):
with E = exp(latent[:, :8]) and S = sum_j E,
  S*z[b,i,h] = sum_j (d1[i,h] + 0.1*s1[h]*A_hat[i,j]) * E[b,j]
so each hidden tile is a K=8 matmul of E^T against a host-folded constant,
then ReLU, then a K=2048 matmul against c[i]*W2, then a 1/S per-row scale
(b2 rides along as an extra K=8 matmul contributing S[b]*b2[o]).

Sharding: pure data-parallel, batch 8192 -> 8 cores x 1024.
"""

import os
import numpy as np

B, NNODE, FDIM, HDIM, ODIM = 8192, 8, 512, 256, 128
NCORES = 8
BC = B // NCORES          # 1024 batch rows per core
NCHUNK = 16               # h-chunks: partitions hold p = i*16 + (h % 16), h = cc*16 + p%16
HALF = 512                # fp32 matmul max moving free dim
KPAD = 32                 # contraction dim for the K=8 matmuls, zero-padded to 32

_CACHE = {}
LAST_RESULTS = None       # BassKernelResults of the most recent run (for profiling)


def _build_nc():
    import concourse.bacc as bacc
    import concourse.mybir as mybir
    from concourse.tile import TileContext
    from concourse._compat import get_trn_type

    fp32 = mybir.dt.float32
    bf16 = mybir.dt.bfloat16
    AF = mybir.ActivationFunctionType

    nc = bacc.Bacc(get_trn_type() or "TRN2", target_bir_lowering=False, debug=True)

    d_latT = nc.dram_tensor("latT", [NNODE, BC], fp32, kind="ExternalInput")
    d_lat8 = nc.dram_tensor("lat8", [BC, NNODE], fp32, kind="ExternalInput")
    d_zlhs = nc.dram_tensor("zlhs", [128, NCHUNK // 4, 128], bf16, kind="ExternalInput")
    d_w2pk = nc.dram_tensor("w2pk", [128, NCHUNK, ODIM], bf16, kind="ExternalInput")
    d_b2r = nc.dram_tensor("b2r", [KPAD, ODIM], bf16, kind="ExternalInput")
    d_out = nc.dram_tensor("out", [BC, ODIM], fp32, kind="ExternalOutput")

    with TileContext(nc) as tc:
        with (
            tc.tile_pool(name="consts", bufs=1) as consts,
            tc.tile_pool(name="work", bufs=1) as work,
            tc.tile_pool(name="hbuf", bufs=1) as hbuf,
            tc.tile_pool(name="outsb", bufs=3) as outsb,
            tc.tile_pool(name="wpsum", bufs=1, space="PSUM") as wpsum,
            tc.tile_pool(name="zpsum", bufs=5, space="PSUM") as zpsum,
            tc.tile_pool(name="opsum", bufs=2, space="PSUM") as opsum,
        ):
            # ---- PE warmup: dense junk matmuls while input DMAs land, to
            # trip the HAM clock gate to 8/8 (2.4 GHz) before real work ----
            wm_lhs = work.tile([KPAD, 32], bf16)
            nc.gpsimd.memset(wm_lhs[:], 0.0)
            wm_rhs = work.tile([KPAD, 256], bf16)
            nc.gpsimd.memset(wm_rhs[:], 0.0)
            wm_ps = wpsum.tile([32, 256], fp32)
            for _ in range(10):
                nc.tensor.matmul(wm_ps[:], wm_lhs[:], wm_rhs[:], start=True, stop=True)

            # ---- inputs (latency-critical first) ----
            sb_latT = work.tile([NNODE, BC], fp32)
            nc.sync.dma_start(out=sb_latT[:], in_=d_latT[:])
            # batch-on-partition copy: [128, t, j] with b = t*128 + p
            sb_lat8 = work.tile([128, BC // 128, NNODE], fp32)
            nc.sync.dma_start(
                out=sb_lat8[:], in_=d_lat8.rearrange("(t p) j -> p t j", p=128)
            )
            sb_zlhs = consts.tile([128, NCHUNK // 4, 128], bf16)
            nc.sync.dma_start(out=sb_zlhs[:], in_=d_zlhs[:])
            sb_b2r = consts.tile([KPAD, ODIM], bf16)
            nc.sync.dma_start(out=sb_b2r[:], in_=d_b2r[:])
            # big, needed late -> SWDGE so it can't queue ahead of the above
            sb_w2 = consts.tile([128, NCHUNK, ODIM], bf16)
            nc.gpsimd.dma_start(out=sb_w2[:], in_=d_w2pk[:])

            # ---- softmax pieces ----
            # E^T = exp(latT), replicated at partitions 32r..32r+8 for the four
            # row-groups of the packed Z matmuls; other partitions zero.
            # (K is padded to 32: <32-partition contractions fault on HW.)
            sb_ET = work.tile([128, BC], bf16)
            nc.vector.memset(sb_ET[:], 0.0)
            for r in range(4):
                nc.scalar.activation(
                    out=sb_ET[32 * r : 32 * r + NNODE, :], in_=sb_latT[:], func=AF.Exp
                )
            # E2 = exp(lat8)    [128, t, j] ; S2 = sum_j ; R2 = 1/S2   [128, t]
            sb_E2 = work.tile([128, BC // 128, NNODE], fp32)
            nc.scalar.activation(out=sb_E2[:], in_=sb_lat8[:], func=AF.Exp)
            sb_S2 = work.tile([128, BC // 128], fp32)
            nc.vector.reduce_sum(out=sb_S2[:], in_=sb_E2[:], axis=mybir.AxisListType.X)
            sb_R2 = work.tile([128, BC // 128], fp32)
            nc.vector.reciprocal(out=sb_R2[:], in_=sb_S2[:])

            # ---- hidden: Z_cc = zlhs_cc^T @ E^T -> relu -> H_cc  [128, BC] ----
            # cc = 4g + r lives at row-group r, free column g; four chunks run
            # concurrently in the PE array via tile_position row packing.
            h_tiles = [None] * NCHUNK
            for g in range(NCHUNK // 4):
                for half in range(BC // HALF):
                    for r in range(4):
                        cc = 4 * g + r
                        if h_tiles[cc] is None:
                            h_tiles[cc] = hbuf.tile(
                                [128, BC], bf16, tag=f"h{cc}", name=f"h{cc}"
                            )
                        z_ps = zpsum.tile([128, HALF], fp32)
                        nc.tensor.matmul(
                            z_ps[:],
                            sb_zlhs[32 * r : 32 * (r + 1), g, :],
                            sb_ET[32 * r : 32 * (r + 1), half * HALF : (half + 1) * HALF],
                            start=True,
                            stop=True,
                            tile_position=(32 * r, 0),
                        )
                        dst = h_tiles[cc][:, half * HALF : (half + 1) * HALF]
                        if (cc + half) % 2 == 0:
                            nc.scalar.activation(out=dst, in_=z_ps[:], func=AF.Relu)
                        else:
                            nc.vector.tensor_scalar_max(dst, z_ps[:], 0.0)

            # ---- output: out[bq] = (sum_cc H_cc^T[:, bq128].T @ W2_cc + S*b2) * r ----
            for bq in range(BC // 128):
                o_ps = opsum.tile([128, ODIM], fp32)
                for cc in range(NCHUNK):
                    nc.tensor.matmul(
                        o_ps[:],
                        h_tiles[cc][:, bq * 128 : (bq + 1) * 128],
                        sb_w2[:, cc, :],
                        start=(cc == 0),
                        stop=False,
                    )
                # + S[b] * b2[o]
                nc.tensor.matmul(
                    o_ps[:],
                    sb_ET[:KPAD, bq * 128 : (bq + 1) * 128],
                    sb_b2r[:],
                    start=False,
                    stop=True,
                )
                o_sb = outsb.tile([128, ODIM], fp32)
                nc.scalar.activation(
                    out=o_sb[:], in_=o_ps[:], func=AF.Copy,
                    scale=sb_R2[:, bq : bq + 1],
                )
                nc.sync.dma_start(
                    out=d_out[bq * 128 : (bq + 1) * 128, :], in_=o_sb[:]
                )

    nc.finalize()
    return nc


def _host_constants(node_features, edge_attr, W1, b1, W2, b2):
    nf = np.asarray(node_features, np.float32)
    ew = np.asarray(edge_attr, np.float32)[:, 0]
    W1 = np.asarray(W1, np.float32)
    b1 = np.asarray(b1, np.float32)
    W2 = np.asarray(W2, np.float32)
    b2 = np.asarray(b2, np.float32)

    # A_hat = D^-1/2 (A + I) D^-1/2, edges (i, j) for all i != j row-major
    src = np.array([i for i in range(NNODE) for j in range(NNODE) if i != j], np.int64)
    dst = np.array([j for i in range(NNODE) for j in range(NNODE) if i != j], np.int64)
    A = np.zeros((NNODE, NNODE), np.float32)
    A[dst, src] = ew
    A = A + np.eye(NNODE, dtype=np.float32)
    deg = A.sum(axis=1)
    dinv = np.where(deg > 0, deg.astype(np.float32) ** -0.5, 0.0).astype(np.float32)
    A_hat = dinv[:, None] * A * dinv[None, :]

    C1 = nf @ W1                      # [8, 256]
    D1 = A_hat @ C1                   # [8, 256]
    d1 = D1 + b1[None, :]             # [8, 256]
    s1 = W1.sum(axis=0)               # [256]
    cvec = A_hat.mean(axis=0)         # [8]

    p = np.arange(128)
    ip = p // 16                      # node index per partition
    qp = p % 16                       # h sub-index per partition

    import ml_dtypes
    bf16 = ml_dtypes.bfloat16

    # zlhs[j, cc, p] = d1[i(p), h(cc,p)] + 0.1*s1[h(cc,p)]*A_hat[i(p), j],
    # packed for 4x row-tiling: chunk cc = 4g + r at partitions 32r+j, column g
    zlhs = np.zeros((128, NCHUNK // 4, 128), np.float32)
    for cc in range(NCHUNK):
        h = cc * 16 + qp              # [128]
        g, r = cc // 4, cc % 4
        zlhs[32 * r : 32 * r + NNODE, g, :] = (
            d1[ip, h][None, :] + 0.1 * s1[h][None, :] * A_hat[ip, :].T
        )

    # w2pk[p, cc, o] = c[i(p)] * W2[h(cc,p), o]
    w2pk = np.empty((128, NCHUNK, ODIM), np.float32)
    for cc in range(NCHUNK):
        h = cc * 16 + qp
        w2pk[:, cc, :] = cvec[ip][:, None] * W2[h, :]

    b2r = np.zeros((KPAD, ODIM), np.float32)
    b2r[:NNODE, :] = b2[None, :]
    return zlhs.astype(bf16), w2pk.astype(bf16), b2r.astype(bf16)


def kernel(latent_vec, node_features, edge_attr, W1, b1, W2, b2):
    global LAST_RESULTS
    from concourse.bass_utils import run_bass_kernel_spmd

    if "nc" not in _CACHE:
        _CACHE["nc"] = _build_nc()
    nc = _CACHE["nc"]

    zlhs, w2pk, b2r = _host_constants(node_features, edge_attr, W1, b1, W2, b2)

    lat8 = np.ascontiguousarray(np.asarray(latent_vec, np.float32)[:, :NNODE])
    in_maps = []
    for c in range(NCORES):
        sl = lat8[c * BC : (c + 1) * BC]
        in_maps.append({
            "latT": np.ascontiguousarray(sl.T),
            "lat8": np.ascontiguousarray(sl),
            "zlhs": zlhs,
            "w2pk": w2pk,
            "b2r": b2r,
        })

    trace = bool(int(os.environ.get("GNN_TRACE", "0")))
    kwargs = {}
    if trace:
        kwargs["trace"] = True
        kwargs["trace_cores"] = [int(x) for x in os.environ.get("GNN_TRACE_CORES", "0").split(",")]
    res = run_bass_kernel_spmd(nc, in_maps, core_ids=list(range(NCORES)), **kwargs)
    LAST_RESULTS = res

    out = np.concatenate([res.results[c]["out"] for c in range(NCORES)], axis=0)
    return out
